# revision 13
# baseline (speedup 1.0000x reference)
"""Distributed Bass kernel for nn_Generator_9887014715849 (topk_masking).

GCN(3 layers over adj@.[10000x10000]) -> concat -> MLP(BN) -> top-k mask.
Row-sharded across 8 NeuronCores.

v4: uniform 125-row contraction tiles so every adj matmul k-group runs
fp8 DoubleRow (5 groups/rank, no normal-mode tail); whole adj shard
host-packed contiguous-per-partition and SBUF-resident for all 3 layers
(one DMA per rank slot); coalesced startup DMAs (packed weights, one
bias/consts tensor); AllGather halves triggered mid-weight-matmul with
single-DMA gathered reads; every AllReduce replaced by AllGather + tiny
PE sel-matrix reduce; BN stats via bn_stats/bn_aggr; output written as
the is_gt indicator in [1,1250] row layout (mlp_out*(1/mlp_out)==1.0
within 1.2e-7 << 2e-2 budget) so the whole mo125/reciprocal HBM bounce
disappears; top-k threshold = 2 histogram rounds on [-8,8] (bin width
6.1e-5 << observed rank gap 4.2e-4).

Self-contained: hardcodes all shapes; host side preps packed shards
and assembles the output.
"""
import sys

sys.path.insert(0, "/opt/trn_rl_repo")

try:
    # Ensure the NTFF hook holder module exists (bass_utils imports it on
    # the trace path). Never overwrite an already-registered module — the
    # boot shim stores the live profile hook in it.
    from antenv import axon_hooks as _axon_hooks  # noqa: F401
except Exception:
    import types as _types

    _m = _types.ModuleType("antenv.axon_hooks")
    _m._NTFF_PROFILE_HOOK = None
    _m.set_axon_ntff_profile_hook = (
        lambda h: setattr(_m, "_NTFF_PROFILE_HOOK", h))
    _m.get_axon_ntff_profile_hook = lambda: _m._NTFF_PROFILE_HOOK
    try:
        import antenv

        sys.modules["antenv.axon_hooks"] = _m
        antenv.axon_hooks = _m
    except Exception:
        pass

import numpy as np
import ml_dtypes

_F8NP = ml_dtypes.float8_e4m3
import concourse.bacc as bacc
import concourse.mybir as mybir
import concourse.tile as tile
from concourse.bass_utils import run_bass_kernel_spmd

F32 = mybir.dt.float32
F16 = mybir.dt.float16
F8 = mybir.dt.float8e4
ALU = mybir.AluOpType
ACT = mybir.ActivationFunctionType
DR = mybir.MatmulPerfMode.DoubleRow

NC = 8
N_NODES = 10000
R = N_NODES // NC          # rows per core (1250)
T = 125                    # contraction tile rows
NT = 10                    # tiles per rank
NG = 5                     # DoubleRow groups per rank (pairs of tiles)
DT = 512                   # dim_touched
C_GCN = [256, 256, 128]
NIN, H1, H2 = 384, 256, 128
NN_K = 100
ASCALE = 8192.0
BN_EPS = 1e-5
SEARCH_LO, SEARCH_HI = -8.0, 8.0
HBINS = 512
W1 = (SEARCH_HI - SEARCH_LO) / HBINS      # 0.03125
W2 = W1 / HBINS                           # 6.1e-5 << rank gap 4.2e-4

R_CHUNKS = [(0, 512), (512, 512), (1024, 226)]
HTILES_A, HTILES_B = 4, 6     # bounce-half split (tiles 0-3 | 4-9)
GRP_A, GRP_B = 2, 3           # DR groups per half

# consts tensor column map ([128, NCOL] f32)
CB_GB1, CB_GB2, CB_GB3 = 0, 2, 4
CB_LB1, CB_LB2, CB_LB3 = 5, 7, 8
CB_IOTA4 = 9                  # 4 cols: p + 128q
CB_SEL32 = 13                 # 4 cols: rows 0-31, sel[k,m]=1 iff k%4==m
CB_SEL16 = 17                 # 2 cols: rows 0-15, sel[k,m]=1 iff k%2==m
NCOL = 19


def _host_consts(gb1, gb2, gb3, lb1, lb2, lb3):
    c = np.zeros((128, NCOL), np.float32)
    c[:, CB_GB1] = gb1[:128]; c[:, CB_GB1 + 1] = gb1[128:]
    c[:, CB_GB2] = gb2[:128]; c[:, CB_GB2 + 1] = gb2[128:]
    c[:, CB_GB3] = gb3
    c[:, CB_LB1] = lb1[:128]; c[:, CB_LB1 + 1] = lb1[128:]
    c[:, CB_LB2] = lb2
    c[0, CB_LB3] = lb3[0]
    p = np.arange(128, dtype=np.float32)
    for q in range(4):
        c[:, CB_IOTA4 + q] = p + 128.0 * q
    for m in range(4):
        c[0:32, CB_SEL32 + m] = (np.arange(32) % 4 == m)
    for m in range(2):
        c[0:16, CB_SEL16 + m] = (np.arange(16) % 2 == m)
    return c


def _pack_k(w, kt):
    """[K, N] -> [128, kt, N] with row t*128+p at [p, t]."""
    K, N = w.shape
    assert K == kt * 128
    return np.ascontiguousarray(w.reshape(kt, 128, N).transpose(1, 0, 2))


def build():
    nc = bacc.Bacc(None, target_bir_lowering=False, num_devices=NC)

    adjp = nc.dram_tensor("adjp", [T, NC * NG * 2 * R], F8,
                          kind="ExternalInput")
    xg = nc.dram_tensor("xg", [128, 4 * R], F16, kind="ExternalInput")
    xm_d = nc.dram_tensor("xm", [128, 2 * R], F32, kind="ExternalInput")
    gw_d = [nc.dram_tensor(f"gw{i+1}", [128, k, n], F16, kind="ExternalInput")
            for i, (k, n) in enumerate([(4, 256), (2, 256), (2, 128)])]
    lw_d = [nc.dram_tensor(f"lw{i+1}", [128, k, n], F32, kind="ExternalInput")
            for i, (k, n) in enumerate([(3, 256), (2, 128), (1, 1)])]
    cst_d = nc.dram_tensor("cst", [128, NCOL], F32, kind="ExternalInput")
    eye_d = nc.dram_tensor("eye", [128, 128], F32, kind="ExternalInput")
    out_d = nc.dram_tensor("out", [1, R], F32, kind="ExternalOutput")

    # collective bounce buffers (internal DRAM)
    sb_a = [nc.dram_tensor(f"sba{l}", [T, HTILES_A * C_GCN[l]], F8)
            for l in range(3)]
    sb_b = [nc.dram_tensor(f"sbb{l}", [T, HTILES_B * C_GCN[l]], F8)
            for l in range(3)]
    sf_a = [nc.dram_tensor(f"sfa{l}", [NC, T, HTILES_A * C_GCN[l]], F8,
                           addr_space="Shared") for l in range(3)]
    sf_b = [nc.dram_tensor(f"sfb{l}", [NC, T, HTILES_B * C_GCN[l]], F8,
                           addr_space="Shared") for l in range(3)]
    bn_in = [nc.dram_tensor(f"bni{j}", [d, 128], F32)
             for j, d in enumerate([4, 2])]
    bn_out = [nc.dram_tensor(f"bno{j}", [NC, d, 128], F32,
                             addr_space="Shared") for j, d in enumerate([4, 2])]
    hist_in = [nc.dram_tensor(f"hci{r}", [4, 128], F32) for r in range(2)]
    hist_out = [nc.dram_tensor(f"hco{r}", [NC, 4, 128], F32,
                               addr_space="Shared") for r in range(2)]

    rg = [list(range(NC))]

    with tile.TileContext(nc) as tc:
        with (
            tc.tile_pool(name="w", bufs=1) as wp,
            tc.tile_pool(name="big", bufs=1) as bp,
            tc.tile_pool(name="adj", bufs=1) as ap_,
            tc.tile_pool(name="stream", bufs=1) as st,
            tc.tile_pool(name="ps", bufs=1, space="PSUM") as pp,
        ):
            # ---- startup loads (order matters: first wmm needs xg+gw1) ----
            xg_t = bp.tile([128, 4, R], F16, tag="xg")
            nc.sync.dma_start(xg_t[:, :, :], xg.ap().rearrange(
                "p (t r) -> p t r", t=4))
            gw_t = []
            for i, (k, n) in enumerate([(4, 256), (2, 256), (2, 128)]):
                t = wp.tile([128, k, n], F16, tag=f"gw{i}")
                nc.sync.dma_start(t[:, :, :], gw_d[i].ap())
                gw_t.append(t)
            adj_t = []
            for g in range(NC):
                t = ap_.tile([T, NG, 2, R], F8, tag=f"adj_{g}")
                nc.sync.dma_start(
                    t[:, :, :, :],
                    adjp[:, g * NG * 2 * R:(g + 1) * NG * 2 * R].rearrange(
                        "p (b j r) -> p b j r", b=NG, j=2))
                adj_t.append(t)
            xm_t = bp.tile([128, 2, R], F32, tag="xm")
            nc.sync.dma_start(xm_t[:, :, :], xm_d.ap().rearrange(
                "p (t r) -> p t r", t=2))
            lw_t = []
            for i, (k, n) in enumerate([(3, 256), (2, 128), (1, 1)]):
                t = wp.tile([128, k, n], F32, tag=f"lw{i}")
                nc.sync.dma_start(t[:, :, :], lw_d[i].ap())
                lw_t.append(t)
            cst = wp.tile([128, NCOL], F32, tag="cst")
            nc.sync.dma_start(cst[:, :], cst_d[:, :])
            eye_t = wp.tile([128, 128], F32, tag="eye")
            nc.sync.dma_start(eye_t[:], eye_d[:, :])
            ones_row = wp.tile([1, 128], F32, tag="ones_row")
            nc.vector.memset(ones_row[:], 1.0)
            ones_col = wp.tile([128, 1], F32, tag="ones_col")
            nc.vector.memset(ones_col[:], 1.0)
            thr1_t = wp.tile([128, 4], F32, tag="thr1")
            nc.vector.tensor_scalar(thr1_t[:], cst[:, CB_IOTA4:CB_IOTA4 + 4],
                                    W1, SEARCH_LO, op0=ALU.mult, op1=ALU.add)

            # ---- helper: weight matmul -> s_own fp8 -> bounce + AG ----
            def wmm(l, h_slice, nkt, cout):
                s_own = st.tile([128, NT, cout], F8, tag="s_own",
                                name=f"s_own_{l}")
                for t in range(NT):
                    psum = pp.tile([T, cout], F32, tag="pss", bufs=2)
                    for kt in range(nkt):
                        nc.tensor.matmul(
                            psum[:], h_slice(kt, t * T, (t + 1) * T),
                            gw_t[l][:, kt, :], start=(kt == 0),
                            stop=(kt == nkt - 1))
                    nc.scalar.activation(s_own[0:T, t, :], psum[:], ACT.Copy)
                    if t == HTILES_A - 1:
                        nc.sync.dma_start(
                            sb_a[l].ap().rearrange("p (t c) -> p t c", t=HTILES_A),
                            s_own[0:T, 0:HTILES_A, :])
                        nc.gpsimd.collective_compute(
                            "AllGather", ALU.bypass, replica_groups=rg,
                            ins=[sb_a[l].ap().opt()], outs=[sf_a[l].ap().opt()])
                    if t == NT - 1:
                        nc.sync.dma_start(
                            sb_b[l].ap().rearrange("p (t c) -> p t c", t=HTILES_B),
                            s_own[0:T, HTILES_A:NT, :])
                        nc.gpsimd.collective_compute(
                            "AllGather", ALU.bypass, replica_groups=rg,
                            ins=[sb_b[l].ap().opt()], outs=[sf_b[l].ap().opt()])

            # ---- helper: adj matmul (all DoubleRow) ----
            def adj_matmul(l, cout, lname, out_dt):
                c_tiles = [(co, min(128, cout - co))
                           for co in range(0, cout, 128)]
                s_ga = st.tile([T, NC, GRP_A * 2, cout], F8, tag="s_ga",
                               name=f"s_ga_{l}")
                nc.sync.dma_start(
                    s_ga[:, :, :, :],
                    sf_a[l].ap().rearrange("g p (x c) -> p g x c",
                                           x=GRP_A * 2))
                s_gb = st.tile([T, NC, GRP_B * 2, cout], F8, tag="s_gb",
                               name=f"s_gb_{l}")
                nc.sync.dma_start(
                    s_gb[:, :, :, :],
                    sf_b[l].ap().rearrange("g p (x c) -> p g x c",
                                           x=GRP_B * 2))
                h_t = [bp.tile([csz, R], out_dt, tag=f"h_{lname}_{ci}",
                               name=f"h_{lname}_{ci}")
                       for ci, (co, csz) in enumerate(c_tiles)]
                psums = {}
                for ci, (co, csz) in enumerate(c_tiles):
                    for ri in range(len(R_CHUNKS)):
                        psums[(ci, ri)] = pp.tile(
                            [csz, R_CHUNKS[ri][1]], F32, tag=f"pa{ci}{ri}",
                            name=f"pa{ci}{ri}_{lname}")
                n_grp = NC * NG
                ki = 0
                for half in range(2):
                    s_g = s_ga if half == 0 else s_gb
                    nb = GRP_A if half == 0 else GRP_B
                    b0 = 0 if half == 0 else GRP_A
                    for g in range(NC):
                        for bl in range(nb):
                            first = ki == 0
                            last = ki == n_grp - 1
                            for ci, (co, csz) in enumerate(c_tiles):
                                sl = s_g[:, g, 2 * bl:2 * bl + 2,
                                         co:co + csz]
                                for ri, (r0, rw) in enumerate(R_CHUNKS):
                                    nc.tensor.matmul(
                                        psums[(ci, ri)][:], sl,
                                        adj_t[g][:, b0 + bl, :,
                                                 r0:r0 + rw],
                                        start=first, stop=last,
                                        perf_mode=DR)
                            ki += 1
                gbc = {0: CB_GB1, 1: CB_GB2, 2: CB_GB3}[l]
                for ci, (co, csz) in enumerate(c_tiles):
                    for ri, (r0, rw) in enumerate(R_CHUNKS):
                        nc.scalar.activation(
                            h_t[ci][:, r0:r0 + rw], psums[(ci, ri)][:],
                            ACT.Relu, bias=cst[0:csz, gbc + ci:gbc + ci + 1],
                            scale=1.0 / ASCALE)
                return h_t

            # ================= GCN =================
            wmm(0, lambda kt, lo, hi: xg_t[:, kt, lo:hi], 4, C_GCN[0])
            h1 = adj_matmul(0, C_GCN[0], "l0", F16)
            wmm(1, lambda kt, lo, hi: h1[kt][:, lo:hi], 2, C_GCN[1])
            h2 = adj_matmul(1, C_GCN[1], "l1", F16)
            wmm(2, lambda kt, lo, hi: h2[kt][:, lo:hi], 2, C_GCN[2])
            h3 = adj_matmul(2, C_GCN[2], "l2", F32)

            # ================= MLP (fp32) =================
            def hcat_slice(kt, lo, hi):
                if kt == 0:
                    return h3[0][:, lo:hi]
                return xm_t[:, kt - 1, lo:hi]

            def mlp_layer(h_slice, nkt, lwt, cout, lb_base, bn_idx, lname):
                c_tiles = [(co, min(128, cout - co))
                           for co in range(0, cout, 128)]
                cn = len(c_tiles)
                # reuse dead GCN h-tile slots (h_l0 dead after wmm(1),
                # h_l1 dead after wmm(2))
                atag = {"m1": ["h_l0_0", "h_l0_1"], "m2": ["h_l1_0"]}[lname]
                a_t = [bp.tile([csz, R], F32, tag=atag[ci],
                               name=f"a_{lname}_{ci}")
                       for ci, (co, csz) in enumerate(c_tiles)]
                st6 = st.tile([128, cn, 3, 6], F32, tag="st6", bufs=2,
                              name=f"st6_{lname}")
                for ci, (co, csz) in enumerate(c_tiles):
                    for ri, (r0, rw) in enumerate(R_CHUNKS):
                        psum = pp.tile([csz, rw], F32, tag=f"pa{ci}{ri}",
                                       bufs=1, name=f"pm_{lname}{ci}{ri}")
                        for kt in range(nkt):
                            nc.tensor.matmul(
                                psum[:], lwt[:, kt, co:co + csz],
                                h_slice(kt, r0, r0 + rw),
                                start=(kt == 0), stop=(kt == nkt - 1))
                        nc.scalar.activation(
                            a_t[ci][:, r0:r0 + rw], psum[:], ACT.Relu,
                            bias=cst[0:csz, lb_base + ci:lb_base + ci + 1])
                        nc.vector.bn_stats(st6[0:csz, ci, ri, :],
                                           a_t[ci][:, r0:r0 + rw])
                # local (sum, sumsq) per channel -> pack [128, 2cn]
                spack = st.tile([128, 2 * cn], F32, tag="spack", bufs=2,
                                name=f"spack_{lname}")
                mv = st.tile([128, 2], F32, tag="mv", bufs=2,
                             name=f"mv_{lname}")
                m2 = st.tile([128, 1], F32, tag="m2", bufs=2,
                             name=f"m2_{lname}")
                for ci, (co, csz) in enumerate(c_tiles):
                    nc.vector.bn_aggr(mv[0:csz, :], st6[0:csz, ci, :, :])
                    nc.vector.tensor_scalar_mul(
                        spack[0:csz, ci:ci + 1], mv[0:csz, 0:1], float(R))
                    nc.vector.tensor_tensor(
                        m2[0:csz, :], mv[0:csz, 0:1], mv[0:csz, 0:1],
                        op=ALU.mult)
                    nc.vector.tensor_tensor(
                        m2[0:csz, :], mv[0:csz, 1:2], m2[0:csz, :],
                        op=ALU.add)
                    nc.vector.tensor_scalar_mul(
                        spack[0:csz, cn + ci:cn + ci + 1], m2[0:csz, :],
                        float(R))
                ptr = pp.tile([2 * cn, 128], F32, tag="pss", bufs=2)
                nc.tensor.transpose(ptr[:], spack[:], eye_t[:])
                statsT = st.tile([2 * cn, 128], F32, tag="stT", bufs=2)
                nc.vector.tensor_copy(statsT[:], ptr[:])
                nc.sync.dma_start(bn_in[bn_idx][:, :], statsT[:])
                nc.gpsimd.collective_compute(
                    "AllGather", ALU.bypass, replica_groups=rg,
                    ins=[bn_in[bn_idx].ap().opt()],
                    outs=[bn_out[bn_idx].ap().opt()])
                gt = st.tile([NC * 2 * cn, 128], F32, tag="gt", bufs=2,
                             name=f"gt_{lname}")
                nc.sync.dma_start(gt[:, :], bn_out[bn_idx].ap().rearrange(
                    "g d p -> (g d) p"))
                sel_base = CB_SEL32 if cn == 2 else CB_SEL16
                # separate sel-reduces so sums and sqs both land at
                # partition 0 (partition-offset APs are illegal)
                gps_s = pp.tile([cn, 128], F32, tag="pss", bufs=2)
                nc.tensor.matmul(gps_s[:], cst[0:NC * 2 * cn,
                                               sel_base:sel_base + cn],
                                 gt[:, :], start=True, stop=True)
                gps_q = pp.tile([cn, 128], F32, tag="pss", bufs=2)
                nc.tensor.matmul(gps_q[:],
                                 cst[0:NC * 2 * cn,
                                     sel_base + cn:sel_base + 2 * cn],
                                 gt[:, :], start=True, stop=True)
                inv_n = 1.0 / N_NODES
                nm = st.tile([cn, 128], F32, tag="nm", bufs=2,
                             name=f"nm_{lname}")
                nc.vector.tensor_scalar_mul(nm[:, :], gps_s[:], -inv_n)
                mm2 = st.tile([cn, 128], F32, tag="mm2", bufs=2,
                              name=f"mm2_{lname}")
                nc.vector.tensor_tensor(mm2[:, :], nm[:, :], nm[:, :],
                                        op=ALU.mult)
                iv = st.tile([cn, 128], F32, tag="iv", bufs=2,
                             name=f"iv_{lname}")
                nc.vector.scalar_tensor_tensor(
                    iv[:, :], gps_q[:], inv_n, mm2[:, :],
                    op0=ALU.mult, op1=ALU.subtract)
                nc.vector.tensor_scalar_add(iv[:, :], iv[:, :], BN_EPS)
                nc.scalar.activation(iv[:, :], iv[:, :], ACT.Sqrt)
                nc.vector.reciprocal(iv[:, :], iv[:, :])
                nmiv_c = st.tile([128, 2 * cn], F32, tag="nmiv_c", bufs=2,
                                 name=f"nmiv_c_{lname}")
                ptn = pp.tile([128, cn], F32, tag="pss", bufs=2)
                nc.tensor.transpose(ptn[:], nm[:, :], eye_t[0:cn, 0:cn])
                nc.vector.tensor_copy(nmiv_c[:, 0:cn], ptn[:])
                pti = pp.tile([128, cn], F32, tag="pss", bufs=2)
                nc.tensor.transpose(pti[:], iv[:, :], eye_t[0:cn, 0:cn])
                nc.vector.tensor_copy(nmiv_c[:, cn:2 * cn], pti[:])
                return a_t, nmiv_c, cn

            a1, nmiv1, cn1 = mlp_layer(hcat_slice, 3, lw_t[0], H1, CB_LB1,
                                       0, "m1")
            for ci in range(cn1):
                nc.vector.tensor_scalar(
                    a1[ci][:], a1[ci][:], nmiv1[:, ci:ci + 1],
                    nmiv1[:, cn1 + ci:cn1 + ci + 1],
                    op0=ALU.add, op1=ALU.mult)
            a2, nmiv2, _ = mlp_layer(lambda kt, lo, hi: a1[kt][:, lo:hi],
                                     2, lw_t[1], H2, CB_LB2, 1, "m2")

            # fold BN2 into lW3: mo = a2 @ (iv2*w3) + ((nm2*iv2)@w3 + b3)
            w3p = st.tile([128, 1], F32, tag="w3p", bufs=2)
            nc.vector.tensor_tensor(w3p[:], nmiv2[:, 1:2], lw_t[2][:, 0, :],
                                    op=ALU.mult)
            nmw = st.tile([128, 1], F32, tag="nmw", bufs=2)
            nc.vector.tensor_tensor(nmw[:], nmiv2[:, 0:1], w3p[:],
                                    op=ALU.mult)
            pnb = pp.tile([1, 1], F32, tag="pss", bufs=2)
            nc.tensor.matmul(pnb[:], nmw[:], ones_col[:], start=True,
                             stop=True)
            nb = st.tile([1, 1], F32, tag="nb", bufs=2)
            nc.vector.tensor_tensor(nb[:], pnb[:], cst[0:1, CB_LB3:CB_LB3 + 1],
                                    op=ALU.add)

            mo = bp.tile([1, R], F32, tag="mo")
            mo_rep = bp.tile([128, R], F32, tag="h_l1_1", name="mo_rep")
            for r0, rw in R_CHUNKS:
                psum = pp.tile([1, rw], F32, tag="pss", bufs=2)
                nc.tensor.matmul(psum[:], w3p[:], a2[0][:, r0:r0 + rw],
                                 start=True, stop=True)
                nc.vector.tensor_scalar(mo[:, r0:r0 + rw], psum[:],
                                        nb[:], None, op0=ALU.add)
                pb = pp.tile([128, rw], F32, tag="pss", bufs=2)
                nc.tensor.matmul(pb[:], ones_row[:], mo[:, r0:r0 + rw],
                                 start=True, stop=True)
                nc.scalar.activation(mo_rep[:, r0:r0 + rw], pb[:], ACT.Copy)

            # ---- distributed top-k threshold: 2 rounds of 512-bin
            # histogram + count AllGather + local PE reduce ----
            def hist_round(rnd, w, lo_col):
                if lo_col is None:
                    thr = thr1_t
                else:
                    thr = st.tile([128, 4], F32, tag="thr", bufs=2)
                    nc.vector.tensor_scalar(
                        thr[:], cst[:, CB_IOTA4:CB_IOTA4 + 4], w, lo_col[:],
                        op0=ALU.mult, op1=ALU.add)
                cnt = st.tile([128, 4], F32, tag="hcnt", bufs=2)
                cmp = st.tile([128, R], F32, tag="hcmp", bufs=1)
                for q in range(4):
                    nc.vector.tensor_scalar(
                        cmp[:], mo_rep[:], thr[:, q:q + 1], 0.0,
                        op0=ALU.is_gt, op1=ALU.add,
                        accum_out=cnt[:, q:q + 1])
                ptr = pp.tile([4, 128], F32, tag="pss", bufs=2)
                nc.tensor.transpose(ptr[:], cnt[:], eye_t[:])
                cntT = st.tile([4, 128], F32, tag="cntT", bufs=2)
                nc.vector.tensor_copy(cntT[:], ptr[:])
                nc.sync.dma_start(hist_in[rnd][:, :], cntT[:])
                nc.gpsimd.collective_compute(
                    "AllGather", ALU.bypass, replica_groups=rg,
                    ins=[hist_in[rnd].ap().opt()],
                    outs=[hist_out[rnd].ap().opt()])
                ght = st.tile([NC * 4, 128], F32, tag="ght", bufs=2)
                nc.sync.dma_start(ght[:, :], hist_out[rnd].ap().rearrange(
                    "g d p -> (g d) p"))
                gps = pp.tile([4, 128], F32, tag="pss", bufs=2)
                nc.tensor.matmul(gps[:], cst[0:32, CB_SEL32:CB_SEL32 + 4],
                                 ght[:, :], start=True, stop=True)
                gcnt = st.tile([4, 128], F32, tag="gcnt", bufs=2)
                nc.vector.tensor_copy(gcnt[:], gps[:])
                selb = st.tile([4, 128], F32, tag="selb", bufs=2)
                rowc = st.tile([4, 1], F32, tag="rowc", bufs=2)
                nc.vector.tensor_scalar(selb[:], gcnt[:], float(NN_K) + 0.5,
                                        0.0, op0=ALU.is_gt, op1=ALU.add,
                                        accum_out=rowc[:])
                pj = pp.tile([1, 1], F32, tag="pss", bufs=2)
                nc.tensor.matmul(pj[:], rowc[:], ones_col[:4, :],
                                 start=True, stop=True)
                jstar = st.tile([1, 1], F32, tag="jst", bufs=2)
                nc.vector.tensor_copy(jstar[:], pj[:])
                return jstar

            js1 = hist_round(0, W1, None)
            lo2 = st.tile([1, 1], F32, tag="lo2", bufs=2)
            nc.vector.tensor_scalar(lo2[:], js1[:], W1, SEARCH_LO - W1,
                                    op0=ALU.mult, op1=ALU.add)
            pl = pp.tile([128, 1], F32, tag="pss", bufs=2)
            nc.tensor.matmul(pl[:], ones_row[:], lo2[:], start=True,
                             stop=True)
            lo2_col = st.tile([128, 1], F32, tag="lo2c", bufs=2)
            nc.vector.tensor_copy(lo2_col[:], pl[:])
            js2 = hist_round(1, W2, lo2_col)
            thr_t = st.tile([1, 1], F32, tag="thrt", bufs=2)
            nc.vector.scalar_tensor_tensor(thr_t[:], js2[:], W2, lo2[:],
                                           op0=ALU.mult, op1=ALU.add)
            # out = 1.0 where mo > thr else 0.0  (== mo * (1/mo) masked)
            sel = st.tile([1, R], F32, tag="selout", bufs=1)
            nc.vector.tensor_scalar(sel[:], mo[:], thr_t[:], None,
                                    op0=ALU.is_gt)
            nc.sync.dma_start(out_d[:, :], sel[:])

    nc.finalize()
    return nc


_NC_CACHE = None


def _get_nc():
    global _NC_CACHE
    if _NC_CACHE is None:
        _NC_CACHE = build()
    return _NC_CACHE


def _prep_core_inputs(x, adj, weights):
    """Host-side shard prep. Returns list of per-core in_maps."""
    in_maps = []
    eye = np.eye(128, dtype=np.float32)
    for i in range(NC):
        rows = slice(i * R, (i + 1) * R)
        adjt = (np.ascontiguousarray(adj[rows, :].T)
                * np.float32(ASCALE)).astype(_F8NP)     # [10000, R]
        # pack: [T, NC, NG, 2, R]: [p, g, b, j] = row g*R + b*250 + j*T + p
        adjp = np.ascontiguousarray(
            adjt.reshape(NC, NG, 2, T, R).transpose(3, 0, 1, 2, 4)
        ).reshape(T, NC * NG * 2 * R)
        xgs = np.ascontiguousarray(x[rows, :DT].T).astype(np.float16)
        xms = np.ascontiguousarray(x[rows, DT:].T)
        m = {
            "adjp": adjp,
            "xg": _pack_k(xgs, 4).reshape(128, 4 * R),
            "xm": _pack_k(xms, 2).reshape(128, 2 * R),
            "eye": eye,
        }
        m.update(weights)
        in_maps.append(m)
    return in_maps


def kernel(x, adj, gW1, gb1, gW2, gb2, gW3, gb3,
           lW1, lb1, lW2, lb2, lW3, lb3, dim_touched, NN,
           _want_result_obj=False, _trace=False):
    x = np.asarray(x, dtype=np.float32)
    adj = np.asarray(adj, dtype=np.float32)
    weights = {
        "gw1": _pack_k(np.asarray(gW1, np.float16), 4),
        "gw2": _pack_k(np.asarray(gW2, np.float16), 2),
        "gw3": _pack_k(np.asarray(gW3, np.float16), 2),
        "lw1": _pack_k(np.asarray(lW1, np.float32), 3),
        "lw2": _pack_k(np.asarray(lW2, np.float32), 2),
        "lw3": _pack_k(np.asarray(lW3, np.float32), 1),
        "cst": _host_consts(np.asarray(gb1, np.float32),
                            np.asarray(gb2, np.float32),
                            np.asarray(gb3, np.float32),
                            np.asarray(lb1, np.float32),
                            np.asarray(lb2, np.float32),
                            np.asarray(lb3, np.float32)),
    }
    in_maps = _prep_core_inputs(x, adj, weights)
    nc = _get_nc()
    res = run_bass_kernel_spmd(nc, in_maps, core_ids=list(range(NC)),
                               trace=_trace)
    out = np.concatenate(
        [res.results[i]["out"].reshape(R) for i in range(NC)]
    ).reshape(N_NODES, 1).astype(np.float32)
    if _want_result_obj:
        return out, res
    return out


# revision 14
# speedup vs baseline: 1.0537x; 1.0537x over previous
"""Distributed Bass kernel for nn_Generator_9887014715849 (topk_masking).

GCN(3 layers over adj@.[10000x10000]) -> concat -> MLP(BN) -> top-k mask.
Row-sharded across 8 NeuronCores.

v4: uniform 125-row contraction tiles so every adj matmul k-group runs
fp8 DoubleRow (5 groups/rank, no normal-mode tail); whole adj shard
host-packed contiguous-per-partition and SBUF-resident for all 3 layers
(one DMA per rank slot); coalesced startup DMAs (packed weights, one
bias/consts tensor); AllGather halves triggered mid-weight-matmul with
single-DMA gathered reads; every AllReduce replaced by AllGather + tiny
PE sel-matrix reduce; BN stats via bn_stats/bn_aggr; output written as
the is_gt indicator in [1,1250] row layout (mlp_out*(1/mlp_out)==1.0
within 1.2e-7 << 2e-2 budget) so the whole mo125/reciprocal HBM bounce
disappears; top-k threshold = 2 histogram rounds on [-8,8] (bin width
6.1e-5 << observed rank gap 4.2e-4).

Self-contained: hardcodes all shapes; host side preps packed shards
and assembles the output.
"""
import sys

sys.path.insert(0, "/opt/trn_rl_repo")

try:
    # Ensure the NTFF hook holder module exists (bass_utils imports it on
    # the trace path). Never overwrite an already-registered module — the
    # boot shim stores the live profile hook in it.
    from antenv import axon_hooks as _axon_hooks  # noqa: F401
except Exception:
    import types as _types

    _m = _types.ModuleType("antenv.axon_hooks")
    _m._NTFF_PROFILE_HOOK = None
    _m.set_axon_ntff_profile_hook = (
        lambda h: setattr(_m, "_NTFF_PROFILE_HOOK", h))
    _m.get_axon_ntff_profile_hook = lambda: _m._NTFF_PROFILE_HOOK
    try:
        import antenv

        sys.modules["antenv.axon_hooks"] = _m
        antenv.axon_hooks = _m
    except Exception:
        pass

import numpy as np
import ml_dtypes

_F8NP = ml_dtypes.float8_e4m3
import concourse.bacc as bacc
import concourse.mybir as mybir
import concourse.tile as tile
from concourse.bass_utils import run_bass_kernel_spmd

F32 = mybir.dt.float32
F16 = mybir.dt.float16
F8 = mybir.dt.float8e4
ALU = mybir.AluOpType
ACT = mybir.ActivationFunctionType
DR = mybir.MatmulPerfMode.DoubleRow

NC = 8
N_NODES = 10000
R = N_NODES // NC          # rows per core (1250)
T = 125                    # contraction tile rows
NT = 10                    # tiles per rank
NG = 5                     # DoubleRow groups per rank (pairs of tiles)
DT = 512                   # dim_touched
C_GCN = [256, 256, 128]
NIN, H1, H2 = 384, 256, 128
NN_K = 100
ASCALE = 8192.0
BN_EPS = 1e-5
SEARCH_LO, SEARCH_HI = -8.0, 8.0
HBINS = 512
W1 = (SEARCH_HI - SEARCH_LO) / HBINS      # 0.03125
W2 = W1 / HBINS                           # 6.1e-5 << rank gap 4.2e-4

R_CHUNKS = [(0, 512), (512, 512), (1024, 226)]
HTILES_A, HTILES_B = 4, 6     # bounce-half split (tiles 0-3 | 4-9)
GRP_A, GRP_B = 2, 3           # DR groups per half

# consts tensor column map ([128, NCOL] f32)
CB_GB1, CB_GB2, CB_GB3 = 0, 2, 4
CB_LB1, CB_LB2, CB_LB3 = 5, 7, 8
CB_IOTA4 = 9                  # 4 cols: p + 128q
CB_SEL32 = 13                 # 4 cols: rows 0-31, sel[k,m]=1 iff k%4==m
CB_SEL16 = 17                 # 2 cols: rows 0-15, sel[k,m]=1 iff k%2==m
NCOL = 19


def _host_consts(gb1, gb2, gb3, lb1, lb2, lb3):
    c = np.zeros((128, NCOL), np.float32)
    c[:, CB_GB1] = gb1[:128]; c[:, CB_GB1 + 1] = gb1[128:]
    c[:, CB_GB2] = gb2[:128]; c[:, CB_GB2 + 1] = gb2[128:]
    c[:, CB_GB3] = gb3
    c[:, CB_LB1] = lb1[:128]; c[:, CB_LB1 + 1] = lb1[128:]
    c[:, CB_LB2] = lb2
    c[0, CB_LB3] = lb3[0]
    p = np.arange(128, dtype=np.float32)
    for q in range(4):
        c[:, CB_IOTA4 + q] = p + 128.0 * q
    for m in range(4):
        c[0:32, CB_SEL32 + m] = (np.arange(32) % 4 == m)
    for m in range(2):
        c[0:16, CB_SEL16 + m] = (np.arange(16) % 2 == m)
    return c


def _pack_k(w, kt):
    """[K, N] -> [128, kt, N] with row t*128+p at [p, t]."""
    K, N = w.shape
    assert K == kt * 128
    return np.ascontiguousarray(w.reshape(kt, 128, N).transpose(1, 0, 2))


def build():
    nc = bacc.Bacc(None, target_bir_lowering=False, num_devices=NC)

    adjp = nc.dram_tensor("adjp", [T, NC * NG * 2 * R], F8,
                          kind="ExternalInput")
    xg = nc.dram_tensor("xg", [128, 4 * R], F16, kind="ExternalInput")
    xm_d = nc.dram_tensor("xm", [128, 2 * R], F32, kind="ExternalInput")
    gw_d = [nc.dram_tensor(f"gw{i+1}", [128, k, n], F16, kind="ExternalInput")
            for i, (k, n) in enumerate([(4, 256), (2, 256), (2, 128)])]
    lw_d = [nc.dram_tensor(f"lw{i+1}", [128, k, n], F32, kind="ExternalInput")
            for i, (k, n) in enumerate([(3, 256), (2, 128), (1, 1)])]
    cst_d = nc.dram_tensor("cst", [128, NCOL], F32, kind="ExternalInput")
    eye_d = nc.dram_tensor("eye", [128, 128], F32, kind="ExternalInput")
    out_d = nc.dram_tensor("out", [1, R], F32, kind="ExternalOutput")

    # collective bounce buffers (internal DRAM)
    sb_a = [nc.dram_tensor(f"sba{l}", [T, HTILES_A * C_GCN[l]], F8)
            for l in range(3)]
    sb_b = [nc.dram_tensor(f"sbb{l}", [T, HTILES_B * C_GCN[l]], F8)
            for l in range(3)]
    sf_a = [nc.dram_tensor(f"sfa{l}", [NC, T, HTILES_A * C_GCN[l]], F8,
                           addr_space="Shared") for l in range(3)]
    sf_b = [nc.dram_tensor(f"sfb{l}", [NC, T, HTILES_B * C_GCN[l]], F8,
                           addr_space="Shared") for l in range(3)]
    bn_in = [nc.dram_tensor(f"bni{j}", [d, 128], F32)
             for j, d in enumerate([4, 2])]
    bn_out = [nc.dram_tensor(f"bno{j}", [NC, d, 128], F32,
                             addr_space="Shared") for j, d in enumerate([4, 2])]
    hist_in = [nc.dram_tensor(f"hci{r}", [4, 128], F32) for r in range(2)]
    hist_out = [nc.dram_tensor(f"hco{r}", [NC, 4, 128], F32,
                               addr_space="Shared") for r in range(2)]

    rg = [list(range(NC))]

    with tile.TileContext(nc) as tc:
        with (
            tc.tile_pool(name="w", bufs=1) as wp,
            tc.tile_pool(name="big", bufs=1) as bp,
            tc.tile_pool(name="adj", bufs=1) as ap_,
            tc.tile_pool(name="stream", bufs=1) as st,
            tc.tile_pool(name="ps", bufs=1, space="PSUM") as pp,
        ):
            # ---- startup loads (order matters: first wmm needs xg+gw1) ----
            xg_t = bp.tile([128, 4, R], F16, tag="xg")
            nc.sync.dma_start(xg_t[:, :, :], xg.ap().rearrange(
                "p (t r) -> p t r", t=4))
            gw_t = []
            for i, (k, n) in enumerate([(4, 256), (2, 256), (2, 128)]):
                t = wp.tile([128, k, n], F16, tag=f"gw{i}")
                nc.sync.dma_start(t[:, :, :], gw_d[i].ap())
                gw_t.append(t)
            adj_t = []
            for g in range(NC):
                t = ap_.tile([T, NG, 2, R], F8, tag=f"adj_{g}")
                nc.sync.dma_start(
                    t[:, :, :, :],
                    adjp[:, g * NG * 2 * R:(g + 1) * NG * 2 * R].rearrange(
                        "p (b j r) -> p b j r", b=NG, j=2))
                adj_t.append(t)
            xm_t = bp.tile([128, 2, R], F32, tag="xm")
            nc.sync.dma_start(xm_t[:, :, :], xm_d.ap().rearrange(
                "p (t r) -> p t r", t=2))
            lw_t = []
            for i, (k, n) in enumerate([(3, 256), (2, 128), (1, 1)]):
                t = wp.tile([128, k, n], F32, tag=f"lw{i}")
                nc.sync.dma_start(t[:, :, :], lw_d[i].ap())
                lw_t.append(t)
            cst = wp.tile([128, NCOL], F32, tag="cst")
            nc.sync.dma_start(cst[:, :], cst_d[:, :])
            eye_t = wp.tile([128, 128], F32, tag="eye")
            nc.sync.dma_start(eye_t[:], eye_d[:, :])
            ones_row = wp.tile([1, 128], F32, tag="ones_row")
            nc.vector.memset(ones_row[:], 1.0)
            ones_col = wp.tile([128, 1], F32, tag="ones_col")
            nc.vector.memset(ones_col[:], 1.0)
            thr1_t = wp.tile([128, 4], F32, tag="thr1")
            nc.vector.tensor_scalar(thr1_t[:], cst[:, CB_IOTA4:CB_IOTA4 + 4],
                                    W1, SEARCH_LO, op0=ALU.mult, op1=ALU.add)

            # ---- helper: weight matmul -> s_own fp8 -> bounce + AG ----
            def wmm(l, h_slice, nkt, cout):
                s_own = st.tile([128, NT, cout], F8, tag="s_own",
                                name=f"s_own_{l}")
                for t in range(NT):
                    psum = pp.tile([T, cout], F32, tag="pss", bufs=2)
                    for kt in range(nkt):
                        nc.tensor.matmul(
                            psum[:], h_slice(kt, t * T, (t + 1) * T),
                            gw_t[l][:, kt, :], start=(kt == 0),
                            stop=(kt == nkt - 1))
                    nc.scalar.activation(s_own[0:T, t, :], psum[:], ACT.Copy)
                    if t == HTILES_A - 1:
                        nc.scalar.dma_start(
                            sb_a[l].ap().rearrange("p (t c) -> p t c", t=HTILES_A),
                            s_own[0:T, 0:HTILES_A, :])
                        nc.gpsimd.collective_compute(
                            "AllGather", ALU.bypass, replica_groups=rg,
                            ins=[sb_a[l].ap().opt()], outs=[sf_a[l].ap().opt()])
                    if t == NT - 1:
                        nc.scalar.dma_start(
                            sb_b[l].ap().rearrange("p (t c) -> p t c", t=HTILES_B),
                            s_own[0:T, HTILES_A:NT, :])
                        nc.gpsimd.collective_compute(
                            "AllGather", ALU.bypass, replica_groups=rg,
                            ins=[sb_b[l].ap().opt()], outs=[sf_b[l].ap().opt()])

            # ---- helper: adj matmul (all DoubleRow) ----
            def adj_matmul(l, cout, lname, out_dt):
                c_tiles = [(co, min(128, cout - co))
                           for co in range(0, cout, 128)]
                s_ga = st.tile([T, NC, GRP_A * 2, cout], F8, tag="s_ga",
                               name=f"s_ga_{l}")
                nc.scalar.dma_start(
                    s_ga[:, :, :, :],
                    sf_a[l].ap().rearrange("g p (x c) -> p g x c",
                                           x=GRP_A * 2))
                s_gb = st.tile([T, NC, GRP_B * 2, cout], F8, tag="s_gb",
                               name=f"s_gb_{l}")
                nc.scalar.dma_start(
                    s_gb[:, :, :, :],
                    sf_b[l].ap().rearrange("g p (x c) -> p g x c",
                                           x=GRP_B * 2))
                h_t = [bp.tile([csz, R], out_dt, tag=f"h_{lname}_{ci}",
                               name=f"h_{lname}_{ci}")
                       for ci, (co, csz) in enumerate(c_tiles)]
                psums = {}
                for ci, (co, csz) in enumerate(c_tiles):
                    for ri in range(len(R_CHUNKS)):
                        psums[(ci, ri)] = pp.tile(
                            [csz, R_CHUNKS[ri][1]], F32, tag=f"pa{ci}{ri}",
                            name=f"pa{ci}{ri}_{lname}")
                n_grp = NC * NG
                ki = 0
                for half in range(2):
                    s_g = s_ga if half == 0 else s_gb
                    nb = GRP_A if half == 0 else GRP_B
                    b0 = 0 if half == 0 else GRP_A
                    for g in range(NC):
                        for bl in range(nb):
                            first = ki == 0
                            last = ki == n_grp - 1
                            for ci, (co, csz) in enumerate(c_tiles):
                                sl = s_g[:, g, 2 * bl:2 * bl + 2,
                                         co:co + csz]
                                for ri, (r0, rw) in enumerate(R_CHUNKS):
                                    nc.tensor.matmul(
                                        psums[(ci, ri)][:], sl,
                                        adj_t[g][:, b0 + bl, :,
                                                 r0:r0 + rw],
                                        start=first, stop=last,
                                        perf_mode=DR)
                            ki += 1
                gbc = {0: CB_GB1, 1: CB_GB2, 2: CB_GB3}[l]
                for ci, (co, csz) in enumerate(c_tiles):
                    for ri, (r0, rw) in enumerate(R_CHUNKS):
                        nc.scalar.activation(
                            h_t[ci][:, r0:r0 + rw], psums[(ci, ri)][:],
                            ACT.Relu, bias=cst[0:csz, gbc + ci:gbc + ci + 1],
                            scale=1.0 / ASCALE)
                return h_t

            # ================= GCN =================
            wmm(0, lambda kt, lo, hi: xg_t[:, kt, lo:hi], 4, C_GCN[0])
            h1 = adj_matmul(0, C_GCN[0], "l0", F16)
            wmm(1, lambda kt, lo, hi: h1[kt][:, lo:hi], 2, C_GCN[1])
            h2 = adj_matmul(1, C_GCN[1], "l1", F16)
            wmm(2, lambda kt, lo, hi: h2[kt][:, lo:hi], 2, C_GCN[2])
            h3 = adj_matmul(2, C_GCN[2], "l2", F32)

            # ================= MLP (fp32) =================
            def hcat_slice(kt, lo, hi):
                if kt == 0:
                    return h3[0][:, lo:hi]
                return xm_t[:, kt - 1, lo:hi]

            def mlp_layer(h_slice, nkt, lwt, cout, lb_base, bn_idx, lname):
                c_tiles = [(co, min(128, cout - co))
                           for co in range(0, cout, 128)]
                cn = len(c_tiles)
                # reuse dead GCN h-tile slots (h_l0 dead after wmm(1),
                # h_l1 dead after wmm(2))
                atag = {"m1": ["h_l0_0", "h_l0_1"], "m2": ["h_l1_0"]}[lname]
                a_t = [bp.tile([csz, R], F32, tag=atag[ci],
                               name=f"a_{lname}_{ci}")
                       for ci, (co, csz) in enumerate(c_tiles)]
                st6 = st.tile([128, cn, 3, 6], F32, tag="st6", bufs=2,
                              name=f"st6_{lname}")
                for ci, (co, csz) in enumerate(c_tiles):
                    for ri, (r0, rw) in enumerate(R_CHUNKS):
                        psum = pp.tile([csz, rw], F32, tag=f"pa{ci}{ri}",
                                       bufs=1, name=f"pm_{lname}{ci}{ri}")
                        for kt in range(nkt):
                            nc.tensor.matmul(
                                psum[:], lwt[:, kt, co:co + csz],
                                h_slice(kt, r0, r0 + rw),
                                start=(kt == 0), stop=(kt == nkt - 1))
                        nc.scalar.activation(
                            a_t[ci][:, r0:r0 + rw], psum[:], ACT.Relu,
                            bias=cst[0:csz, lb_base + ci:lb_base + ci + 1])
                        nc.vector.bn_stats(st6[0:csz, ci, ri, :],
                                           a_t[ci][:, r0:r0 + rw])
                # local (sum, sumsq) per channel -> pack [128, 2cn]
                spack = st.tile([128, 2 * cn], F32, tag="spack", bufs=2,
                                name=f"spack_{lname}")
                mv = st.tile([128, 2], F32, tag="mv", bufs=2,
                             name=f"mv_{lname}")
                m2 = st.tile([128, 1], F32, tag="m2", bufs=2,
                             name=f"m2_{lname}")
                for ci, (co, csz) in enumerate(c_tiles):
                    nc.vector.bn_aggr(mv[0:csz, :], st6[0:csz, ci, :, :])
                    nc.vector.tensor_scalar_mul(
                        spack[0:csz, ci:ci + 1], mv[0:csz, 0:1], float(R))
                    nc.vector.tensor_tensor(
                        m2[0:csz, :], mv[0:csz, 0:1], mv[0:csz, 0:1],
                        op=ALU.mult)
                    nc.vector.tensor_tensor(
                        m2[0:csz, :], mv[0:csz, 1:2], m2[0:csz, :],
                        op=ALU.add)
                    nc.vector.tensor_scalar_mul(
                        spack[0:csz, cn + ci:cn + ci + 1], m2[0:csz, :],
                        float(R))
                ptr = pp.tile([2 * cn, 128], F32, tag="pss", bufs=2)
                nc.tensor.transpose(ptr[:], spack[:], eye_t[:])
                statsT = st.tile([2 * cn, 128], F32, tag="stT", bufs=2)
                nc.vector.tensor_copy(statsT[:], ptr[:])
                nc.scalar.dma_start(bn_in[bn_idx][:, :], statsT[:])
                nc.gpsimd.collective_compute(
                    "AllGather", ALU.bypass, replica_groups=rg,
                    ins=[bn_in[bn_idx].ap().opt()],
                    outs=[bn_out[bn_idx].ap().opt()])
                gt = st.tile([NC * 2 * cn, 128], F32, tag="gt", bufs=2,
                             name=f"gt_{lname}")
                nc.scalar.dma_start(gt[:, :], bn_out[bn_idx].ap().rearrange(
                    "g d p -> (g d) p"))
                sel_base = CB_SEL32 if cn == 2 else CB_SEL16
                # separate sel-reduces so sums and sqs both land at
                # partition 0 (partition-offset APs are illegal)
                gps_s = pp.tile([cn, 128], F32, tag="pss", bufs=2)
                nc.tensor.matmul(gps_s[:], cst[0:NC * 2 * cn,
                                               sel_base:sel_base + cn],
                                 gt[:, :], start=True, stop=True)
                gps_q = pp.tile([cn, 128], F32, tag="pss", bufs=2)
                nc.tensor.matmul(gps_q[:],
                                 cst[0:NC * 2 * cn,
                                     sel_base + cn:sel_base + 2 * cn],
                                 gt[:, :], start=True, stop=True)
                inv_n = 1.0 / N_NODES
                nm = st.tile([cn, 128], F32, tag="nm", bufs=2,
                             name=f"nm_{lname}")
                nc.vector.tensor_scalar_mul(nm[:, :], gps_s[:], -inv_n)
                mm2 = st.tile([cn, 128], F32, tag="mm2", bufs=2,
                              name=f"mm2_{lname}")
                nc.vector.tensor_tensor(mm2[:, :], nm[:, :], nm[:, :],
                                        op=ALU.mult)
                iv = st.tile([cn, 128], F32, tag="iv", bufs=2,
                             name=f"iv_{lname}")
                nc.vector.scalar_tensor_tensor(
                    iv[:, :], gps_q[:], inv_n, mm2[:, :],
                    op0=ALU.mult, op1=ALU.subtract)
                nc.vector.tensor_scalar_add(iv[:, :], iv[:, :], BN_EPS)
                nc.scalar.activation(iv[:, :], iv[:, :], ACT.Sqrt)
                nc.vector.reciprocal(iv[:, :], iv[:, :])
                nmiv_c = st.tile([128, 2 * cn], F32, tag="nmiv_c", bufs=2,
                                 name=f"nmiv_c_{lname}")
                ptn = pp.tile([128, cn], F32, tag="pss", bufs=2)
                nc.tensor.transpose(ptn[:], nm[:, :], eye_t[0:cn, 0:cn])
                nc.vector.tensor_copy(nmiv_c[:, 0:cn], ptn[:])
                pti = pp.tile([128, cn], F32, tag="pss", bufs=2)
                nc.tensor.transpose(pti[:], iv[:, :], eye_t[0:cn, 0:cn])
                nc.vector.tensor_copy(nmiv_c[:, cn:2 * cn], pti[:])
                return a_t, nmiv_c, cn

            a1, nmiv1, cn1 = mlp_layer(hcat_slice, 3, lw_t[0], H1, CB_LB1,
                                       0, "m1")
            for ci in range(cn1):
                nc.vector.tensor_scalar(
                    a1[ci][:], a1[ci][:], nmiv1[:, ci:ci + 1],
                    nmiv1[:, cn1 + ci:cn1 + ci + 1],
                    op0=ALU.add, op1=ALU.mult)
            a2, nmiv2, _ = mlp_layer(lambda kt, lo, hi: a1[kt][:, lo:hi],
                                     2, lw_t[1], H2, CB_LB2, 1, "m2")

            # fold BN2 into lW3: mo = a2 @ (iv2*w3) + ((nm2*iv2)@w3 + b3)
            w3p = st.tile([128, 1], F32, tag="w3p", bufs=2)
            nc.vector.tensor_tensor(w3p[:], nmiv2[:, 1:2], lw_t[2][:, 0, :],
                                    op=ALU.mult)
            nmw = st.tile([128, 1], F32, tag="nmw", bufs=2)
            nc.vector.tensor_tensor(nmw[:], nmiv2[:, 0:1], w3p[:],
                                    op=ALU.mult)
            pnb = pp.tile([1, 1], F32, tag="pss", bufs=2)
            nc.tensor.matmul(pnb[:], nmw[:], ones_col[:], start=True,
                             stop=True)
            nb = st.tile([1, 1], F32, tag="nb", bufs=2)
            nc.vector.tensor_tensor(nb[:], pnb[:], cst[0:1, CB_LB3:CB_LB3 + 1],
                                    op=ALU.add)

            mo = bp.tile([1, R], F32, tag="mo")
            mo_rep = bp.tile([128, R], F32, tag="h_l1_1", name="mo_rep")
            for r0, rw in R_CHUNKS:
                psum = pp.tile([1, rw], F32, tag="pss", bufs=2)
                nc.tensor.matmul(psum[:], w3p[:], a2[0][:, r0:r0 + rw],
                                 start=True, stop=True)
                nc.vector.tensor_scalar(mo[:, r0:r0 + rw], psum[:],
                                        nb[:], None, op0=ALU.add)
                pb = pp.tile([128, rw], F32, tag="pss", bufs=2)
                nc.tensor.matmul(pb[:], ones_row[:], mo[:, r0:r0 + rw],
                                 start=True, stop=True)
                nc.scalar.activation(mo_rep[:, r0:r0 + rw], pb[:], ACT.Copy)

            # ---- distributed top-k threshold: 2 rounds of 512-bin
            # histogram + count AllGather + local PE reduce ----
            def hist_round(rnd, w, lo_col):
                if lo_col is None:
                    thr = thr1_t
                else:
                    thr = st.tile([128, 4], F32, tag="thr", bufs=2)
                    nc.vector.tensor_scalar(
                        thr[:], cst[:, CB_IOTA4:CB_IOTA4 + 4], w, lo_col[:],
                        op0=ALU.mult, op1=ALU.add)
                cnt = st.tile([128, 4], F32, tag="hcnt", bufs=2)
                cmp = st.tile([128, R], F32, tag="hcmp", bufs=1)
                for q in range(4):
                    nc.vector.tensor_scalar(
                        cmp[:], mo_rep[:], thr[:, q:q + 1], 0.0,
                        op0=ALU.is_gt, op1=ALU.add,
                        accum_out=cnt[:, q:q + 1])
                ptr = pp.tile([4, 128], F32, tag="pss", bufs=2)
                nc.tensor.transpose(ptr[:], cnt[:], eye_t[:])
                cntT = st.tile([4, 128], F32, tag="cntT", bufs=2)
                nc.vector.tensor_copy(cntT[:], ptr[:])
                nc.scalar.dma_start(hist_in[rnd][:, :], cntT[:])
                nc.gpsimd.collective_compute(
                    "AllGather", ALU.bypass, replica_groups=rg,
                    ins=[hist_in[rnd].ap().opt()],
                    outs=[hist_out[rnd].ap().opt()])
                ght = st.tile([NC * 4, 128], F32, tag="ght", bufs=2)
                nc.scalar.dma_start(ght[:, :], hist_out[rnd].ap().rearrange(
                    "g d p -> (g d) p"))
                gps = pp.tile([4, 128], F32, tag="pss", bufs=2)
                nc.tensor.matmul(gps[:], cst[0:32, CB_SEL32:CB_SEL32 + 4],
                                 ght[:, :], start=True, stop=True)
                gcnt = st.tile([4, 128], F32, tag="gcnt", bufs=2)
                nc.vector.tensor_copy(gcnt[:], gps[:])
                selb = st.tile([4, 128], F32, tag="selb", bufs=2)
                rowc = st.tile([4, 1], F32, tag="rowc", bufs=2)
                nc.vector.tensor_scalar(selb[:], gcnt[:], float(NN_K) + 0.5,
                                        0.0, op0=ALU.is_gt, op1=ALU.add,
                                        accum_out=rowc[:])
                pj = pp.tile([1, 1], F32, tag="pss", bufs=2)
                nc.tensor.matmul(pj[:], rowc[:], ones_col[:4, :],
                                 start=True, stop=True)
                jstar = st.tile([1, 1], F32, tag="jst", bufs=2)
                nc.vector.tensor_copy(jstar[:], pj[:])
                return jstar

            js1 = hist_round(0, W1, None)
            lo2 = st.tile([1, 1], F32, tag="lo2", bufs=2)
            nc.vector.tensor_scalar(lo2[:], js1[:], W1, SEARCH_LO - W1,
                                    op0=ALU.mult, op1=ALU.add)
            pl = pp.tile([128, 1], F32, tag="pss", bufs=2)
            nc.tensor.matmul(pl[:], ones_row[:], lo2[:], start=True,
                             stop=True)
            lo2_col = st.tile([128, 1], F32, tag="lo2c", bufs=2)
            nc.vector.tensor_copy(lo2_col[:], pl[:])
            js2 = hist_round(1, W2, lo2_col)
            thr_t = st.tile([1, 1], F32, tag="thrt", bufs=2)
            nc.vector.scalar_tensor_tensor(thr_t[:], js2[:], W2, lo2[:],
                                           op0=ALU.mult, op1=ALU.add)
            # out = 1.0 where mo > thr else 0.0  (== mo * (1/mo) masked)
            sel = st.tile([1, R], F32, tag="selout", bufs=1)
            nc.vector.tensor_scalar(sel[:], mo[:], thr_t[:], None,
                                    op0=ALU.is_gt)
            nc.scalar.dma_start(out_d[:, :], sel[:])

    nc.finalize()
    return nc


_NC_CACHE = None


def _get_nc():
    global _NC_CACHE
    if _NC_CACHE is None:
        _NC_CACHE = build()
    return _NC_CACHE


def _prep_core_inputs(x, adj, weights):
    """Host-side shard prep. Returns list of per-core in_maps."""
    in_maps = []
    eye = np.eye(128, dtype=np.float32)
    for i in range(NC):
        rows = slice(i * R, (i + 1) * R)
        adjt = (np.ascontiguousarray(adj[rows, :].T)
                * np.float32(ASCALE)).astype(_F8NP)     # [10000, R]
        # pack: [T, NC, NG, 2, R]: [p, g, b, j] = row g*R + b*250 + j*T + p
        adjp = np.ascontiguousarray(
            adjt.reshape(NC, NG, 2, T, R).transpose(3, 0, 1, 2, 4)
        ).reshape(T, NC * NG * 2 * R)
        xgs = np.ascontiguousarray(x[rows, :DT].T).astype(np.float16)
        xms = np.ascontiguousarray(x[rows, DT:].T)
        m = {
            "adjp": adjp,
            "xg": _pack_k(xgs, 4).reshape(128, 4 * R),
            "xm": _pack_k(xms, 2).reshape(128, 2 * R),
            "eye": eye,
        }
        m.update(weights)
        in_maps.append(m)
    return in_maps


def kernel(x, adj, gW1, gb1, gW2, gb2, gW3, gb3,
           lW1, lb1, lW2, lb2, lW3, lb3, dim_touched, NN,
           _want_result_obj=False, _trace=False):
    x = np.asarray(x, dtype=np.float32)
    adj = np.asarray(adj, dtype=np.float32)
    weights = {
        "gw1": _pack_k(np.asarray(gW1, np.float16), 4),
        "gw2": _pack_k(np.asarray(gW2, np.float16), 2),
        "gw3": _pack_k(np.asarray(gW3, np.float16), 2),
        "lw1": _pack_k(np.asarray(lW1, np.float32), 3),
        "lw2": _pack_k(np.asarray(lW2, np.float32), 2),
        "lw3": _pack_k(np.asarray(lW3, np.float32), 1),
        "cst": _host_consts(np.asarray(gb1, np.float32),
                            np.asarray(gb2, np.float32),
                            np.asarray(gb3, np.float32),
                            np.asarray(lb1, np.float32),
                            np.asarray(lb2, np.float32),
                            np.asarray(lb3, np.float32)),
    }
    in_maps = _prep_core_inputs(x, adj, weights)
    nc = _get_nc()
    res = run_bass_kernel_spmd(nc, in_maps, core_ids=list(range(NC)),
                               trace=_trace)
    out = np.concatenate(
        [res.results[i]["out"].reshape(R) for i in range(NC)]
    ).reshape(N_NODES, 1).astype(np.float32)
    if _want_result_obj:
        return out, res
    return out
